# revision 33
# baseline (speedup 1.0000x reference)
"""Trainium2 Bass kernel for nn_DP_44315472560786 (DeePMD descriptor + derivative).

Self-contained: hardcodes shapes/sharding. Data-parallel over frames: 8 cores
x 4 frames. Host packs per-element planes into [C=4, 128, W=400] tiles
(partition = atom-row mod 128, free = (row-slot r, neighbor m)); row slot r has
atom type r%2, so per-type stats are pre-concatenated along the free dim and
every on-chip op is a full-width contiguous [128, 400] op.

All five f32 input planes ride in ONE DMA per chunk ([128, 5W] slab), mask in a
second (u8), and all 16 output channel planes leave in ONE [128, 16W] DMA —
few, large transfers, and consumers are given dedicated "touch" instructions so
no DVE tensor op ever needs more than one valued semaphore wait (the DVE ISA
structs can't encode more).

Math (validated vs reference at ~2e-7 / 2e-6 rel): with du = 1/(RMAX-RMIN),
uc = clamp01((x-RMIN)*du) = relu(1 - relu((RMAX-x)*du)); the quintic switch
vv = (((-6*uc+15)*uc-10)*uc^3+1)*mask and its derivative enter via
  C = dvv*inr*mask, H = inr^2*vv, E = H-C, U = inr^2*(H+E), S0 = inr*rstd0*E
  Ri0 = inr*rstd0*vv - davg0*rstd0
  Rij = dRj*rstdj*inr^2*(vv+1-mask) - davgj*rstdj
  Rd[0,k] = dRk*S0 ;  Rd[j,k] = (dRj*rstdj*U)*dRk - delta_jk*H*rstdj
"""
import os
import sys

for _p in ("/opt/trn_rl_repo", "/root/.axon_site/_ro/trn_rl_repo"):
    if os.path.isdir(_p):
        sys.path.insert(0, _p)
        break

import numpy as np

import concourse.bass as bass
import concourse.mybir as mybir
from concourse.tile import TileContext
from concourse.bass_utils import run_bass_kernel_spmd

F32 = mybir.dt.float32
U8 = mybir.dt.uint8
AF = mybir.ActivationFunctionType
OP = mybir.AluOpType

B, N, M = 32, 256, 200
NCORES = 8
FPC = B // NCORES            # frames per core
R = 2                        # row-slots per partition per chunk (= types 0,1)
C = FPC * N // (R * 128)     # chunks per core (4)
W = R * M                    # free width (400)
ROWS = C * R * 128           # atom-rows per core (1024)
NCH = 16                     # output channels: ri0..3, d00..d23

RMAX = 6.0
DU = 5.0                     # 1/(RMAX-RMIN) rounded to f32

OUT_NAMES = ["ri0", "ri1", "ri2", "ri3"] + [f"d{j}{k}" for j in range(4) for k in range(3)]

LAST_RESULT = None           # BassKernelResults of the most recent run (for test.py)


def _dp_core_kernel(tc, outs, ins, n_chunks, w):
    nc = tc.nc

    with tc.tile_pool(name="pstat", bufs=1) as pstat, \
         tc.tile_pool(name="ptch", bufs=8) as ptch, \
         tc.tile_pool(name="pin", bufs=n_chunks) as pin, \
         tc.tile_pool(name="pmid", bufs=2) as pmid, \
         tc.tile_pool(name="ppsum", bufs=2, space="PSUM") as ppsum, \
         tc.tile_pool(name="pout", bufs=2) as pout:

        def bias_const(val, nm):
            t = pstat.tile([128, 1], F32, tag=nm, name=nm)
            nc.vector.memset(t[:], val)
            return t

        b30 = bias_const(float(RMAX * DU), "b30")
        b1 = bias_const(1.0, "b1")
        bm125 = bias_const(-1.25, "bm125")
        bm0625 = bias_const(-0.625, "bm0625")
        # [1,128] row of -1s: rank-1 lhsT for the -davg*rstd PSUM accumulate
        cneg = pstat.tile([1, 128], F32, tag="cneg", name="cneg")
        nc.vector.memset(cneg[:], -1.0)

        # "touch" reads: a dedicated first consumer per (DMA'd tile, engine) so
        # real compute ops never carry more than one valued semaphore wait.
        # Rotating slots (bufs=8) keep the touches themselves wait-free on WAW.
        def vtouch(ap, nm):
            t = ptch.tile([128, 1], F32, tag="vt", name=f"vt_{nm}")
            nc.vector.tensor_copy(out=t[:], in_=ap[:, :1])

        def atouch(ap, nm):
            t = ptch.tile([128, 1], F32, tag="at", name=f"at_{nm}")
            nc.scalar.copy(out=t[:], in_=ap[:, :1])

        st = pstat.tile([128, 8 * w + 128], F32, tag="stats", name="stats")
        nc.sync.dma_start(out=st[:], in_=ins["stats"])
        vtouch(st, "st")
        rstdf = [st[:, c * w:(c + 1) * w] for c in range(4)]
        darf = [st[:, (4 + c) * w:(5 + c) * w] for c in range(4)]
        ident = st[:, 8 * w:8 * w + 128]

        for ic in range(n_chunks):
            inb = pin.tile([128, 5 * w], F32, tag="inb", name=f"inb_{ic}")
            nc.sync.dma_start(out=inb[:], in_=ins["inb"][ic])
            tmsk = pin.tile([128, w], U8, tag="msk", name=f"msk_{ic}")
            nc.sync.dma_start(out=tmsk[:], in_=ins["msk"][ic])
            vtouch(inb, f"inb{ic}")
            atouch(inb, f"inb{ic}")
            atouch(tmsk, f"msk{ic}")

            dR = [inb[:, 0:w], inb[:, w:2 * w], inb[:, 2 * w:3 * w]]
            tx = inb[:, 3 * w:4 * w]
            tinr = inb[:, 4 * w:5 * w]

            # one output slab; channel ch = outb[:, ch*w:(ch+1)*w]
            outb = pout.tile([128, NCH * w], F32, tag="outb", name=f"outb_{ic}")
            # DVE write-touch carries the WAR wait on the slab's previous
            # out-DMA so real writers and the out-DMA stay single-wait.
            nc.vector.tensor_copy(out=outb[:, 0:1], in_=b1[:])
            och = {nm: outb[:, i * w:(i + 1) * w] for i, nm in enumerate(OUT_NAMES)}

            def mid(tag):
                return pmid.tile([128, w], F32, tag=tag, name=f"{tag}_{ic}")

            mf = mid("mf")
            nc.scalar.copy(out=mf[:], in_=tmsk[:])
            aa = mid("aa")
            nc.scalar.activation(out=aa[:], in_=tx, func=AF.Relu,
                                 bias=b30[:], scale=float(-DU))
            uc = mid("uc")
            nc.scalar.activation(out=uc[:], in_=aa[:], func=AF.Relu,
                                 bias=b1[:], scale=-1.0)
            u2 = mid("u2")
            nc.scalar.square(out=u2[:], in_=uc[:])
            i2 = mid("i2")
            nc.scalar.square(out=i2[:], in_=tinr)
            # Q = (uc - 1.25)^2 ; the quintic's quadratic factor enters as
            # qb - 10 = -6*Q - 0.625, with constants folded into the fused ops
            Q = mid("Q")
            nc.scalar.activation(out=Q[:], in_=uc[:], func=AF.Square,
                                 bias=bm125[:], scale=1.0)
            # Q2 = (uc - 0.625)^2 ; w/3 = -4*Q2 + 1.5625
            Q2 = mid("Q2")
            nc.scalar.activation(out=Q2[:], in_=uc[:], func=AF.Square,
                                 bias=bm0625[:], scale=1.0)

            pa1 = mid("pa1")  # (Q + 0.625/6)*uc
            nc.vector.scalar_tensor_tensor(out=pa1[:], in0=Q[:], scalar=float(0.625 / 6),
                                           in1=uc[:], op0=OP.add, op1=OP.mult)
            pa = mid("pa")    # -6*pa1*u2 = (qb-10)*uc*u2
            nc.vector.scalar_tensor_tensor(out=pa[:], in0=pa1[:], scalar=-6.0,
                                           in1=u2[:], op0=OP.mult, op1=OP.mult)
            vv = mid("vv")
            nc.vector.scalar_tensor_tensor(out=vv[:], in0=pa[:], scalar=1.0,
                                           in1=mf[:], op0=OP.add, op1=OP.mult)
            W1 = mid("W1")
            nc.vector.scalar_tensor_tensor(out=W1[:], in0=vv[:], scalar=1.0,
                                           in1=mf[:], op0=OP.add, op1=OP.subtract)
            t1 = mid("t1")    # 1.5*Q + Q2, so q + w/3 = -4*(t1 - 0.234375)
            nc.vector.scalar_tensor_tensor(out=t1[:], in0=Q[:], scalar=1.5,
                                           in1=Q2[:], op0=OP.mult, op1=OP.add)
            dv = mid("dv")    # (t1 - 0.234375)*u2 = -(q + w/3)*u2/4
            nc.vector.scalar_tensor_tensor(out=dv[:], in0=t1[:], scalar=0.234375,
                                           in1=u2[:], op0=OP.subtract, op1=OP.mult)
            im = mid("im"); nc.vector.tensor_mul(out=im[:], in0=tinr, in1=mf[:])
            Ct = mid("Ct")    # dvv*inr_m = (-4*3*du)*dv*im
            nc.vector.scalar_tensor_tensor(out=Ct[:], in0=dv[:], scalar=float(-12 * DU),
                                           in1=im[:], op0=OP.mult, op1=OP.mult)
            H = mid("H"); nc.vector.tensor_mul(out=H[:], in0=i2[:], in1=vv[:])
            E = mid("E"); nc.vector.tensor_sub(out=E[:], in0=H[:], in1=Ct[:])
            Ft = mid("Ft"); nc.vector.tensor_add(out=Ft[:], in0=H[:], in1=E[:])
            U = mid("U"); nc.vector.tensor_mul(out=U[:], in0=i2[:], in1=Ft[:])
            V = mid("V"); nc.vector.tensor_mul(out=V[:], in0=i2[:], in1=W1[:])

            inrr0 = mid("inrr0")
            nc.vector.tensor_mul(out=inrr0[:], in0=tinr, in1=rstdf[0])
            dRs = []
            for j in range(3):
                t = mid(f"dRs{j}")
                nc.vector.tensor_mul(out=t[:], in0=dR[j], in1=rstdf[j + 1])
                dRs.append(t)
            S0 = mid("S0"); nc.vector.tensor_mul(out=S0[:], in0=inrr0[:], in1=E[:])

            # Ri channels: DVE writes the raw product to a mid tile, PE runs
            # identity @ raw then accumulates the rank-1 (-1s) x (davg*rstd)
            # term in PSUM, ACT copies PSUM -> output slab.
            for c in range(4):
                rw = mid(f"rw{c}")
                if c == 0:
                    nc.vector.tensor_mul(out=rw[:], in0=inrr0[:], in1=vv[:])
                else:
                    nc.vector.tensor_mul(out=rw[:], in0=dRs[c - 1][:], in1=V[:])
                pt = ppsum.tile([128, w], F32, tag=f"ps{c}", name=f"ps{c}_{ic}")
                nc.tensor.matmul(pt[:], ident, rw[:], start=True, stop=False)
                nc.tensor.matmul(pt[:], cneg[:], st[0:1, (4 + c) * w:(5 + c) * w],
                                 start=False, stop=True)
                nc.scalar.copy(out=och[f"ri{c}"], in_=pt[:])

            for k in range(3):
                nc.vector.tensor_mul(out=och[f"d0{k}"], in0=dR[k], in1=S0[:])

            for j in range(3):
                Aj = mid(f"A{j}")
                nc.vector.tensor_mul(out=Aj[:], in0=dRs[j][:], in1=U[:])
                hr = mid("hr")
                nc.vector.tensor_mul(out=hr[:], in0=H[:], in1=rstdf[j + 1])
                for k in range(3):
                    o = och[f"d{j+1}{k}"]
                    nc.vector.tensor_mul(out=o, in0=Aj[:], in1=dR[k])
                    if k == j:
                        nc.vector.tensor_sub(out=o, in0=o, in1=hr[:])

            nc.sync.dma_start(out=outs["outb"][ic], in_=outb[:])


def _split_multiwaits(nc):
    """Walrus codegen can encode only one valued semaphore wait per
    instruction (the EVENTS semaphore_value field is shared). Tile sometimes
    emits more. Move extra valued waits onto injected same-engine NoOps placed
    immediately before the instruction — engines execute in order, so waiting
    earlier on the same queue is semantically identical."""
    skip = ("InstEventSemaphore",)
    for fn in nc.m.functions:
        for bb in fn.blocks:
            newlist = []
            changed = False
            for ins in bb.instructions:
                si = ins.sync_info
                if si is not None and type(ins).__name__ not in skip:
                    waits = list(si.on_wait or [])
                    valued = [w for w in waits if w.wait_value is not None]
                    if len(valued) > 1:
                        keep = valued[-1]
                        unvalued = [w for w in waits if w.wait_value is None]
                        for w in valued[:-1]:
                            nop = mybir.InstNoOp(
                                name=nc.get_next_instruction_name(), ins=[], outs=[])
                            nop.engine = ins.engine
                            nop.sync_info = mybir.SyncInfo(on_wait=[w], on_update=[])
                            newlist.append(nop)
                        ins.sync_info = mybir.SyncInfo(
                            on_wait=unvalued + [keep],
                            on_update=list(si.on_update or []))
                        changed = True
                newlist.append(ins)
            if changed:
                bb.instructions[:] = newlist


_NC = {}


def _build(split=True):
    if split in _NC:
        return _NC[split]
    nc = bass.Bass("TRN2", target_bir_lowering=False, debug=False)
    ins = {
        "inb": nc.dram_tensor("inb", [C, 128, 5 * W], F32, kind="ExternalInput").ap(),
        "msk": nc.dram_tensor("msk", [C, 128, W], U8, kind="ExternalInput").ap(),
        "stats": nc.dram_tensor("stats", [128, 8 * W + 128], F32, kind="ExternalInput").ap(),
    }
    outs = {
        "outb": nc.dram_tensor("outb", [C, 128, NCH * W], F32, kind="ExternalOutput").ap(),
    }
    with TileContext(nc) as tc:
        _dp_core_kernel(tc, outs, ins, C, W)
    if split:
        _split_multiwaits(nc)
    _NC[split] = nc
    return nc


def _build_timed(iters):
    """Variant that repeats the whole per-core workload `iters` times in a
    hardware For_i loop — used by the timing probe to isolate device exec
    time from dispatch/transfer overhead."""
    nc = bass.Bass("TRN2", target_bir_lowering=False, debug=False)
    ins = {
        "inb": nc.dram_tensor("inb", [C, 128, 5 * W], F32, kind="ExternalInput").ap(),
        "msk": nc.dram_tensor("msk", [C, 128, W], U8, kind="ExternalInput").ap(),
        "stats": nc.dram_tensor("stats", [128, 8 * W + 128], F32, kind="ExternalInput").ap(),
    }
    outs = {
        "outb": nc.dram_tensor("outb", [C, 128, NCH * W], F32, kind="ExternalOutput").ap(),
    }
    with TileContext(nc) as tc:
        with tc.For_i(0, iters, 1):
            _dp_core_kernel(tc, outs, ins, C, W)
    _split_multiwaits(nc)
    return nc


def _pack_plane(plane):
    """[ROWS, M] -> [C, 128, W]."""
    return plane.reshape(C, R, 128, M).transpose(0, 2, 1, 3).reshape(C, 128, W)


def _unpack_plane(packed):
    """[C, 128, W] -> [ROWS, M]."""
    return packed.reshape(C, 128, R, M).transpose(0, 2, 1, 3).reshape(ROWS, M)


def _pack_stats(davg, dstd):
    """davg/dstd [2, 800] -> [128, 8W] (rstd c=0..3 then dar c=0..3; slot r = type r%2)."""
    f32 = np.float32
    davg = davg.reshape(2, M, 4).astype(f32)
    dstd = dstd.reshape(2, M, 4).astype(f32)
    rstd = (f32(1.0) / dstd).astype(f32)
    dar = (davg * rstd).astype(f32)
    planes = np.empty((8, W), dtype=f32)
    for c in range(4):
        for r in range(R):
            t = r % 2
            planes[c, r * M:(r + 1) * M] = rstd[t, :, c]
            planes[4 + c, r * M:(r + 1) * M] = dar[t, :, c]
    out = np.empty((128, 8 * W + 128), dtype=f32)
    out[:, :8 * W] = planes.reshape(1, 8 * W)
    out[:, 8 * W:] = np.eye(128, dtype=f32)
    return np.ascontiguousarray(out)


def kernel(image_dR, x, Ri_xyz, mask, inr, davg, dstd, natoms_per_type):
    global LAST_RESULT
    image_dR = np.asarray(image_dR, dtype=np.float32)
    x = np.asarray(x, dtype=np.float32)
    inr = np.asarray(inr, dtype=np.float32)
    mask_u8 = np.asarray(mask).astype(np.uint8)
    davg = np.asarray(davg, dtype=np.float32)
    dstd = np.asarray(dstd, dtype=np.float32)

    nc = _build()
    stats = _pack_stats(davg, dstd)

    in_maps = []
    for ci in range(NCORES):
        f0 = ci * FPC
        dRc = image_dR[f0:f0 + FPC].reshape(ROWS, M, 3)
        planes = [
            _pack_plane(np.ascontiguousarray(dRc[:, :, 0])),
            _pack_plane(np.ascontiguousarray(dRc[:, :, 1])),
            _pack_plane(np.ascontiguousarray(dRc[:, :, 2])),
            _pack_plane(x[f0:f0 + FPC].reshape(ROWS, M)),
            _pack_plane(inr[f0:f0 + FPC].reshape(ROWS, M)),
        ]
        inb = np.ascontiguousarray(np.concatenate(planes, axis=2))
        in_maps.append({
            "inb": inb,
            "msk": np.ascontiguousarray(_pack_plane(mask_u8[f0:f0 + FPC].reshape(ROWS, M))),
            "stats": stats,
        })

    res = run_bass_kernel_spmd(nc, in_maps, core_ids=list(range(NCORES)))
    LAST_RESULT = res

    Ri = np.empty((B, N, M, 4), dtype=np.float32)
    Ri_d = np.empty((B, N, M, 4, 3), dtype=np.float32)
    for ci in range(NCORES):
        f0 = ci * FPC
        outb = res.results[ci]["outb"]
        for i, nm in enumerate(OUT_NAMES):
            pl = _unpack_plane(outb[:, :, i * W:(i + 1) * W]).reshape(FPC, N, M)
            if nm.startswith("ri"):
                Ri[f0:f0 + FPC, :, :, int(nm[2])] = pl
            else:
                Ri_d[f0:f0 + FPC, :, :, int(nm[1]), int(nm[2])] = pl
    return Ri, Ri_d


# revision 37
# speedup vs baseline: 1.0299x; 1.0299x over previous
"""Trainium2 Bass kernel for nn_DP_44315472560786 (DeePMD descriptor + derivative).

Self-contained: hardcodes shapes/sharding. Data-parallel over frames: 8 cores
x 4 frames. Host packs per-element planes into [C=4, 128, W=400] tiles
(partition = atom-row mod 128, free = (row-slot r, neighbor m)); row slot r has
atom type r%2, so per-type stats are pre-concatenated along the free dim and
every on-chip op is a full-width contiguous [128, 400] op.

All five f32 input planes ride in ONE DMA per chunk ([128, 5W] slab), mask in a
second (u8), and all 16 output channel planes leave in ONE [128, 16W] DMA —
few, large transfers, and consumers are given dedicated "touch" instructions so
no DVE tensor op ever needs more than one valued semaphore wait (the DVE ISA
structs can't encode more).

Math (validated vs reference at ~2e-7 / 2e-6 rel): with du = 1/(RMAX-RMIN),
uc = clamp01((x-RMIN)*du) = relu(1 - relu((RMAX-x)*du)); the quintic switch
vv = (((-6*uc+15)*uc-10)*uc^3+1)*mask and its derivative enter via
  C = dvv*inr*mask, H = inr^2*vv, E = H-C, U = inr^2*(H+E), S0 = inr*rstd0*E
  Ri0 = inr*rstd0*vv - davg0*rstd0
  Rij = dRj*rstdj*inr^2*(vv+1-mask) - davgj*rstdj
  Rd[0,k] = dRk*S0 ;  Rd[j,k] = (dRj*rstdj*U)*dRk - delta_jk*H*rstdj
"""
import os
import sys

for _p in ("/opt/trn_rl_repo", "/root/.axon_site/_ro/trn_rl_repo"):
    if os.path.isdir(_p):
        sys.path.insert(0, _p)
        break

import numpy as np

import concourse.bass as bass
import concourse.mybir as mybir
from concourse.tile import TileContext
from concourse.bass_utils import run_bass_kernel_spmd

F32 = mybir.dt.float32
U8 = mybir.dt.uint8
AF = mybir.ActivationFunctionType
OP = mybir.AluOpType

B, N, M = 32, 256, 200
NCORES = 8
FPC = B // NCORES            # frames per core
R = 2                        # row-slots per partition per chunk (= types 0,1)
C = FPC * N // (R * 128)     # chunks per core (4)
W = R * M                    # free width (400)
ROWS = C * R * 128           # atom-rows per core (1024)
NCH = 16                     # output channels: ri0..3, d00..d23

RMAX = 6.0
DU = 5.0                     # 1/(RMAX-RMIN) rounded to f32

OUT_NAMES = ["ri0", "ri1", "ri2", "ri3"] + [f"d{j}{k}" for j in range(4) for k in range(3)]

LAST_RESULT = None           # BassKernelResults of the most recent run (for test.py)


def _dp_core_kernel(tc, outs, ins, n_chunks, w):
    nc = tc.nc

    with tc.tile_pool(name="pstat", bufs=1) as pstat, \
         tc.tile_pool(name="ptch", bufs=8) as ptch, \
         tc.tile_pool(name="pin", bufs=min(3, n_chunks)) as pin, \
         tc.tile_pool(name="pmid", bufs=2) as pmid, \
         tc.tile_pool(name="ppsum", bufs=2, space="PSUM") as ppsum, \
         tc.tile_pool(name="pout", bufs=2) as pout:

        def bias_const(val, nm):
            t = pstat.tile([128, 1], F32, tag=nm, name=nm)
            nc.vector.memset(t[:], val)
            return t

        b30 = bias_const(float(RMAX * DU), "b30")
        b1 = bias_const(1.0, "b1")
        b5 = bias_const(5.0, "b5")
        bm125 = bias_const(-1.25, "bm125")
        # [1,128] row of -1s: rank-1 lhsT for the -davg*rstd PSUM accumulate
        cneg = pstat.tile([1, 128], F32, tag="cneg", name="cneg")
        nc.vector.memset(cneg[:], -1.0)

        # "touch" reads: a dedicated first consumer per (DMA'd tile, engine) so
        # real compute ops never carry more than one valued semaphore wait.
        # Rotating slots (bufs=8) keep the touches themselves wait-free on WAW.
        def vtouch(ap, nm):
            t = ptch.tile([128, 1], F32, tag="vt", name=f"vt_{nm}")
            nc.vector.tensor_copy(out=t[:], in_=ap[:, :1])

        def atouch(ap, nm):
            t = ptch.tile([128, 1], F32, tag="at", name=f"at_{nm}")
            nc.scalar.copy(out=t[:], in_=ap[:, :1])

        st = pstat.tile([128, 8 * w + 128], F32, tag="stats", name="stats")
        nc.sync.dma_start(out=st[:], in_=ins["stats"])
        vtouch(st, "st")
        rstdf = [st[:, c * w:(c + 1) * w] for c in range(4)]
        darf = [st[:, (4 + c) * w:(5 + c) * w] for c in range(4)]
        ident = st[:, 8 * w:8 * w + 128]

        for ic in range(n_chunks):
            inb = pin.tile([128, 5 * w], F32, tag="inb", name=f"inb_{ic}")
            nc.sync.dma_start(out=inb[:], in_=ins["inb"][ic])
            tmsk = pin.tile([128, w], U8, tag="msk", name=f"msk_{ic}")
            nc.sync.dma_start(out=tmsk[:], in_=ins["msk"][ic])
            vtouch(inb, f"inb{ic}")
            atouch(inb, f"inb{ic}")
            atouch(tmsk, f"msk{ic}")

            dR = [inb[:, 0:w], inb[:, w:2 * w], inb[:, 2 * w:3 * w]]
            tx = inb[:, 3 * w:4 * w]
            tinr = inb[:, 4 * w:5 * w]

            # one output slab; channel ch = outb[:, ch*w:(ch+1)*w]
            outb = pout.tile([128, NCH * w], F32, tag="outb", name=f"outb_{ic}")
            # DVE write-touch carries the WAR wait on the slab's previous
            # out-DMA so real writers and the out-DMA stay single-wait.
            nc.vector.tensor_copy(out=outb[:, 0:1], in_=b1[:])
            och = {nm: outb[:, i * w:(i + 1) * w] for i, nm in enumerate(OUT_NAMES)}

            def mid(tag):
                return pmid.tile([128, w], F32, tag=tag, name=f"{tag}_{ic}")

            mf = mid("mf")
            nc.scalar.copy(out=mf[:], in_=tmsk[:])
            aa = mid("aa")
            nc.scalar.activation(out=aa[:], in_=tx, func=AF.Relu,
                                 bias=b30[:], scale=float(-DU))
            uc = mid("uc")
            nc.scalar.activation(out=uc[:], in_=aa[:], func=AF.Relu,
                                 bias=b1[:], scale=-1.0)
            u2 = mid("u2")
            nc.scalar.square(out=u2[:], in_=uc[:])
            i2 = mid("i2")
            nc.scalar.square(out=i2[:], in_=tinr)
            # Q = (uc - 1.25)^2 ; the quintic's quadratic factor enters as
            # qb - 10 = -6*Q - 0.625, with constants folded into the fused ops
            Q = mid("Q")
            nc.scalar.activation(out=Q[:], in_=uc[:], func=AF.Square,
                                 bias=bm125[:], scale=1.0)
            wa = mid("wa")
            nc.scalar.activation(out=wa[:], in_=uc[:], func=AF.Identity,
                                 bias=b5[:], scale=-4.0)

            pa1 = mid("pa1")  # (Q + 0.625/6)*uc
            nc.vector.scalar_tensor_tensor(out=pa1[:], in0=Q[:], scalar=float(0.625 / 6),
                                           in1=uc[:], op0=OP.add, op1=OP.mult)
            pa = mid("pa")    # -6*pa1*u2 = (qb-10)*uc*u2
            nc.vector.scalar_tensor_tensor(out=pa[:], in0=pa1[:], scalar=-6.0,
                                           in1=u2[:], op0=OP.mult, op1=OP.mult)
            vv = mid("vv")
            nc.vector.scalar_tensor_tensor(out=vv[:], in0=pa[:], scalar=1.0,
                                           in1=mf[:], op0=OP.add, op1=OP.mult)
            W1 = mid("W1")
            nc.vector.scalar_tensor_tensor(out=W1[:], in0=vv[:], scalar=1.0,
                                           in1=mf[:], op0=OP.add, op1=OP.subtract)
            wb = mid("wb"); nc.vector.tensor_mul(out=wb[:], in0=wa[:], in1=uc[:])
            s = mid("s")      # -6*Q + wb = (qb-10) + 0.625 + wb
            nc.vector.scalar_tensor_tensor(out=s[:], in0=Q[:], scalar=-6.0,
                                           in1=wb[:], op0=OP.mult, op1=OP.add)
            dv = mid("dv")    # (s - 0.625)*u2
            nc.vector.scalar_tensor_tensor(out=dv[:], in0=s[:], scalar=0.625,
                                           in1=u2[:], op0=OP.subtract, op1=OP.mult)
            im = mid("im"); nc.vector.tensor_mul(out=im[:], in0=tinr, in1=mf[:])
            Ct = mid("Ct")
            nc.vector.scalar_tensor_tensor(out=Ct[:], in0=dv[:], scalar=float(3 * DU),
                                           in1=im[:], op0=OP.mult, op1=OP.mult)
            H = mid("H"); nc.vector.tensor_mul(out=H[:], in0=i2[:], in1=vv[:])
            E = mid("E"); nc.vector.tensor_sub(out=E[:], in0=H[:], in1=Ct[:])
            Ft = mid("Ft"); nc.vector.tensor_add(out=Ft[:], in0=H[:], in1=E[:])
            U = mid("U"); nc.vector.tensor_mul(out=U[:], in0=i2[:], in1=Ft[:])
            V = mid("V"); nc.vector.tensor_mul(out=V[:], in0=i2[:], in1=W1[:])

            # wide triple helpers: [128,3w] views / free-dim step-0 broadcasts
            def as3(ap3w):
                return ap3w.rearrange("p (o w) -> p o w", o=3)

            def bc3(ap1w):
                return ap1w.rearrange("p (o w) -> p o w", o=1).to_broadcast([128, 3, w])

            dR3 = as3(inb[:, 0:3 * w])
            rstd123 = as3(st[:, w:4 * w])

            inrr0 = mid("inrr0")
            nc.vector.tensor_mul(out=inrr0[:], in0=tinr, in1=rstdf[0])
            dRsw = pmid.tile([128, 3 * w], F32, tag="dRsw", name=f"dRsw_{ic}")
            nc.vector.tensor_mul(out=as3(dRsw[:]), in0=dR3, in1=rstd123)
            dRs = [dRsw[:, j * w:(j + 1) * w] for j in range(3)]
            S0 = mid("S0"); nc.vector.tensor_mul(out=S0[:], in0=inrr0[:], in1=E[:])

            # Ri channels: DVE writes the raw product to a mid tile, PE runs
            # identity @ raw then accumulates the rank-1 (-1s) x (davg*rstd)
            # term in PSUM, ACT copies PSUM -> output slab.
            for c in range(4):
                rw = mid(f"rw{c}")
                if c == 0:
                    nc.vector.tensor_mul(out=rw[:], in0=inrr0[:], in1=vv[:])
                else:
                    nc.vector.tensor_mul(out=rw[:], in0=dRs[c - 1][:], in1=V[:])
                pt = ppsum.tile([128, w], F32, tag=f"ps{c}", name=f"ps{c}_{ic}")
                nc.tensor.matmul(pt[:], ident, rw[:], start=True, stop=False)
                nc.tensor.matmul(pt[:], cneg[:], st[0:1, (4 + c) * w:(5 + c) * w],
                                 start=False, stop=True)
                nc.scalar.copy(out=och[f"ri{c}"], in_=pt[:])

            # Rd row 0: one wide op, S0 broadcast over k
            nc.vector.tensor_mul(out=as3(outb[:, 4 * w:7 * w]), in0=dR3, in1=bc3(S0[:]))

            # A_j = dRs_j * U (U broadcast over j), one wide op
            Aw = pmid.tile([128, 3 * w], F32, tag="Aw", name=f"Aw_{ic}")
            nc.vector.tensor_mul(out=as3(Aw[:]), in0=as3(dRsw[:]), in1=bc3(U[:]))
            # hr_j = H * rstd_j, one wide op
            hrw = pmid.tile([128, 3 * w], F32, tag="hrw", name=f"hrw_{ic}")
            nc.vector.tensor_mul(out=as3(hrw[:]), in0=bc3(H[:]), in1=rstd123)

            for j in range(3):
                # Rd row j+1: A_j broadcast over k against contiguous dR
                blk = outb[:, (7 + 3 * j) * w:(10 + 3 * j) * w]
                nc.vector.tensor_mul(out=as3(blk),
                                     in0=bc3(Aw[:, j * w:(j + 1) * w]), in1=dR3)
                o = och[f"d{j+1}{j}"]
                nc.vector.tensor_sub(out=o, in0=o, in1=hrw[:, j * w:(j + 1) * w])

            nc.sync.dma_start(out=outs["outb"][ic], in_=outb[:])


def _split_multiwaits(nc):
    """Walrus codegen can encode only one valued semaphore wait per
    instruction (the EVENTS semaphore_value field is shared). Tile sometimes
    emits more. Move extra valued waits onto injected same-engine NoOps placed
    immediately before the instruction — engines execute in order, so waiting
    earlier on the same queue is semantically identical."""
    skip = ("InstEventSemaphore",)
    for fn in nc.m.functions:
        for bb in fn.blocks:
            newlist = []
            changed = False
            for ins in bb.instructions:
                si = ins.sync_info
                if si is not None and type(ins).__name__ not in skip:
                    waits = list(si.on_wait or [])
                    valued = [w for w in waits if w.wait_value is not None]
                    if len(valued) > 1:
                        keep = valued[-1]
                        unvalued = [w for w in waits if w.wait_value is None]
                        for w in valued[:-1]:
                            nop = mybir.InstNoOp(
                                name=nc.get_next_instruction_name(), ins=[], outs=[])
                            nop.engine = ins.engine
                            nop.sync_info = mybir.SyncInfo(on_wait=[w], on_update=[])
                            newlist.append(nop)
                        ins.sync_info = mybir.SyncInfo(
                            on_wait=unvalued + [keep],
                            on_update=list(si.on_update or []))
                        changed = True
                newlist.append(ins)
            if changed:
                bb.instructions[:] = newlist


_NC = {}


def _build(split=True):
    if split in _NC:
        return _NC[split]
    nc = bass.Bass("TRN2", target_bir_lowering=False, debug=False)
    ins = {
        "inb": nc.dram_tensor("inb", [C, 128, 5 * W], F32, kind="ExternalInput").ap(),
        "msk": nc.dram_tensor("msk", [C, 128, W], U8, kind="ExternalInput").ap(),
        "stats": nc.dram_tensor("stats", [128, 8 * W + 128], F32, kind="ExternalInput").ap(),
    }
    outs = {
        "outb": nc.dram_tensor("outb", [C, 128, NCH * W], F32, kind="ExternalOutput").ap(),
    }
    with TileContext(nc) as tc:
        _dp_core_kernel(tc, outs, ins, C, W)
    if split:
        _split_multiwaits(nc)
    _NC[split] = nc
    return nc


def _build_timed(iters):
    """Variant that repeats the whole per-core workload `iters` times in a
    hardware For_i loop — used by the timing probe to isolate device exec
    time from dispatch/transfer overhead."""
    nc = bass.Bass("TRN2", target_bir_lowering=False, debug=False)
    ins = {
        "inb": nc.dram_tensor("inb", [C, 128, 5 * W], F32, kind="ExternalInput").ap(),
        "msk": nc.dram_tensor("msk", [C, 128, W], U8, kind="ExternalInput").ap(),
        "stats": nc.dram_tensor("stats", [128, 8 * W + 128], F32, kind="ExternalInput").ap(),
    }
    outs = {
        "outb": nc.dram_tensor("outb", [C, 128, NCH * W], F32, kind="ExternalOutput").ap(),
    }
    with TileContext(nc) as tc:
        with tc.For_i(0, iters, 1):
            _dp_core_kernel(tc, outs, ins, C, W)
    _split_multiwaits(nc)
    return nc


def _pack_plane(plane):
    """[ROWS, M] -> [C, 128, W]."""
    return plane.reshape(C, R, 128, M).transpose(0, 2, 1, 3).reshape(C, 128, W)


def _unpack_plane(packed):
    """[C, 128, W] -> [ROWS, M]."""
    return packed.reshape(C, 128, R, M).transpose(0, 2, 1, 3).reshape(ROWS, M)


def _pack_stats(davg, dstd):
    """davg/dstd [2, 800] -> [128, 8W] (rstd c=0..3 then dar c=0..3; slot r = type r%2)."""
    f32 = np.float32
    davg = davg.reshape(2, M, 4).astype(f32)
    dstd = dstd.reshape(2, M, 4).astype(f32)
    rstd = (f32(1.0) / dstd).astype(f32)
    dar = (davg * rstd).astype(f32)
    planes = np.empty((8, W), dtype=f32)
    for c in range(4):
        for r in range(R):
            t = r % 2
            planes[c, r * M:(r + 1) * M] = rstd[t, :, c]
            planes[4 + c, r * M:(r + 1) * M] = dar[t, :, c]
    out = np.empty((128, 8 * W + 128), dtype=f32)
    out[:, :8 * W] = planes.reshape(1, 8 * W)
    out[:, 8 * W:] = np.eye(128, dtype=f32)
    return np.ascontiguousarray(out)


def kernel(image_dR, x, Ri_xyz, mask, inr, davg, dstd, natoms_per_type):
    global LAST_RESULT
    image_dR = np.asarray(image_dR, dtype=np.float32)
    x = np.asarray(x, dtype=np.float32)
    inr = np.asarray(inr, dtype=np.float32)
    mask_u8 = np.asarray(mask).astype(np.uint8)
    davg = np.asarray(davg, dtype=np.float32)
    dstd = np.asarray(dstd, dtype=np.float32)

    nc = _build()
    stats = _pack_stats(davg, dstd)

    in_maps = []
    for ci in range(NCORES):
        f0 = ci * FPC
        dRc = image_dR[f0:f0 + FPC].reshape(ROWS, M, 3)
        planes = [
            _pack_plane(np.ascontiguousarray(dRc[:, :, 0])),
            _pack_plane(np.ascontiguousarray(dRc[:, :, 1])),
            _pack_plane(np.ascontiguousarray(dRc[:, :, 2])),
            _pack_plane(x[f0:f0 + FPC].reshape(ROWS, M)),
            _pack_plane(inr[f0:f0 + FPC].reshape(ROWS, M)),
        ]
        inb = np.ascontiguousarray(np.concatenate(planes, axis=2))
        in_maps.append({
            "inb": inb,
            "msk": np.ascontiguousarray(_pack_plane(mask_u8[f0:f0 + FPC].reshape(ROWS, M))),
            "stats": stats,
        })

    res = run_bass_kernel_spmd(nc, in_maps, core_ids=list(range(NCORES)))
    LAST_RESULT = res

    Ri = np.empty((B, N, M, 4), dtype=np.float32)
    Ri_d = np.empty((B, N, M, 4, 3), dtype=np.float32)
    for ci in range(NCORES):
        f0 = ci * FPC
        outb = res.results[ci]["outb"]
        for i, nm in enumerate(OUT_NAMES):
            pl = _unpack_plane(outb[:, :, i * W:(i + 1) * W]).reshape(FPC, N, M)
            if nm.startswith("ri"):
                Ri[f0:f0 + FPC, :, :, int(nm[2])] = pl
            else:
                Ri_d[f0:f0 + FPC, :, :, int(nm[1]), int(nm[2])] = pl
    return Ri, Ri_d


# revision 39
# speedup vs baseline: 1.1084x; 1.0763x over previous
"""Trainium2 Bass kernel for nn_DP_44315472560786 (DeePMD descriptor + derivative).

Self-contained: hardcodes shapes/sharding. Data-parallel over frames: 8 cores
x 4 frames. Host packs per-element planes into [C=4, 128, W=400] tiles
(partition = atom-row mod 128, free = (row-slot r, neighbor m)); row slot r has
atom type r%2, so per-type stats are pre-concatenated along the free dim and
every on-chip op is a full-width contiguous [128, 400] op.

All five f32 input planes ride in ONE DMA per chunk ([128, 5W] slab), mask in a
second (u8), and all 16 output channel planes leave in ONE [128, 16W] DMA —
few, large transfers, and consumers are given dedicated "touch" instructions so
no DVE tensor op ever needs more than one valued semaphore wait (the DVE ISA
structs can't encode more).

Math (validated vs reference at ~2e-7 / 2e-6 rel): with du = 1/(RMAX-RMIN),
uc = clamp01((x-RMIN)*du) = relu(1 - relu((RMAX-x)*du)); the quintic switch
vv = (((-6*uc+15)*uc-10)*uc^3+1)*mask and its derivative enter via
  C = dvv*inr*mask, H = inr^2*vv, E = H-C, U = inr^2*(H+E), S0 = inr*rstd0*E
  Ri0 = inr*rstd0*vv - davg0*rstd0
  Rij = dRj*rstdj*inr^2*(vv+1-mask) - davgj*rstdj
  Rd[0,k] = dRk*S0 ;  Rd[j,k] = (dRj*rstdj*U)*dRk - delta_jk*H*rstdj
"""
import os
import sys

for _p in ("/opt/trn_rl_repo", "/root/.axon_site/_ro/trn_rl_repo"):
    if os.path.isdir(_p):
        sys.path.insert(0, _p)
        break

import numpy as np

import concourse.bass as bass
import concourse.mybir as mybir
from concourse.tile import TileContext
from concourse.bass_utils import run_bass_kernel_spmd

F32 = mybir.dt.float32
U8 = mybir.dt.uint8
AF = mybir.ActivationFunctionType
OP = mybir.AluOpType

B, N, M = 32, 256, 200
NCORES = 8
FPC = B // NCORES            # frames per core
R = 2                        # row-slots per partition per chunk (= types 0,1)
C = FPC * N // (R * 128)     # chunks per core (4)
W = R * M                    # free width (400)
ROWS = C * R * 128           # atom-rows per core (1024)
NCH = 16                     # output channels: ri0..3, d00..d23

RMAX = 6.0
DU = 5.0                     # 1/(RMAX-RMIN) rounded to f32

OUT_NAMES = ["ri0", "ri1", "ri2", "ri3"] + [f"d{j}{k}" for j in range(4) for k in range(3)]

LAST_RESULT = None           # BassKernelResults of the most recent run (for test.py)


def _dp_core_kernel(tc, outs, ins, n_chunks, w):
    nc = tc.nc

    with tc.tile_pool(name="pstat", bufs=1) as pstat, \
         tc.tile_pool(name="ptch", bufs=8) as ptch, \
         tc.tile_pool(name="pin", bufs=min(3, n_chunks)) as pin, \
         tc.tile_pool(name="pmid", bufs=2) as pmid, \
         tc.tile_pool(name="ppsum", bufs=2, space="PSUM") as ppsum, \
         tc.tile_pool(name="pout", bufs=2) as pout:

        def bias_const(val, nm):
            t = pstat.tile([128, 1], F32, tag=nm, name=nm)
            nc.vector.memset(t[:], val)
            return t

        b30 = bias_const(float(RMAX * DU), "b30")
        b1 = bias_const(1.0, "b1")
        b5 = bias_const(5.0, "b5")
        bm125 = bias_const(-1.25, "bm125")
        # [1,128] row of -1s: rank-1 lhsT for the -davg*rstd PSUM accumulate
        cneg = pstat.tile([1, 128], F32, tag="cneg", name="cneg")
        nc.vector.memset(cneg[:], -1.0)

        # "touch" reads: a dedicated first consumer per (DMA'd tile, engine) so
        # real compute ops never carry more than one valued semaphore wait.
        # Rotating slots (bufs=8) keep the touches themselves wait-free on WAW.
        def vtouch(ap, nm):
            t = ptch.tile([128, 1], F32, tag="vt", name=f"vt_{nm}")
            nc.vector.tensor_copy(out=t[:], in_=ap[:, :1])

        def atouch(ap, nm):
            t = ptch.tile([128, 1], F32, tag="at", name=f"at_{nm}")
            nc.scalar.copy(out=t[:], in_=ap[:, :1])

        st = pstat.tile([128, 8 * w + 128], F32, tag="stats", name="stats")
        nc.sync.dma_start(out=st[:], in_=ins["stats"])
        vtouch(st, "st")
        rstdf = [st[:, c * w:(c + 1) * w] for c in range(4)]
        darf = [st[:, (4 + c) * w:(5 + c) * w] for c in range(4)]
        ident = st[:, 8 * w:8 * w + 128]

        for ic in range(n_chunks):
            inb = pin.tile([128, 5 * w], F32, tag="inb", name=f"inb_{ic}")
            nc.sync.dma_start(out=inb[:], in_=ins["inb"][ic])
            tmsk = pin.tile([128, w], U8, tag="msk", name=f"msk_{ic}")
            nc.sync.dma_start(out=tmsk[:], in_=ins["msk"][ic])
            vtouch(inb, f"inb{ic}")
            atouch(inb, f"inb{ic}")
            atouch(tmsk, f"msk{ic}")

            dR = [inb[:, 0:w], inb[:, w:2 * w], inb[:, 2 * w:3 * w]]
            tx = inb[:, 3 * w:4 * w]
            tinr = inb[:, 4 * w:5 * w]

            # one output slab; channel ch = outb[:, ch*w:(ch+1)*w]
            outb = pout.tile([128, NCH * w], F32, tag="outb", name=f"outb_{ic}")
            # DVE write-touch carries the WAR wait on the slab's previous
            # out-DMA so real writers and the out-DMA stay single-wait.
            nc.vector.tensor_copy(out=outb[:, 0:1], in_=b1[:])
            och = {nm: outb[:, i * w:(i + 1) * w] for i, nm in enumerate(OUT_NAMES)}

            def mid(tag):
                return pmid.tile([128, w], F32, tag=tag, name=f"{tag}_{ic}")

            mf = mid("mf")
            nc.scalar.copy(out=mf[:], in_=tmsk[:])
            aa = mid("aa")
            nc.scalar.activation(out=aa[:], in_=tx, func=AF.Relu,
                                 bias=b30[:], scale=float(-DU))
            uc = mid("uc")
            nc.scalar.activation(out=uc[:], in_=aa[:], func=AF.Relu,
                                 bias=b1[:], scale=-1.0)
            u2 = mid("u2")
            nc.scalar.square(out=u2[:], in_=uc[:])
            i2 = mid("i2")
            nc.scalar.square(out=i2[:], in_=tinr)
            # Q = (uc - 1.25)^2 ; the quintic's quadratic factor enters as
            # qb - 10 = -6*Q - 0.625, with constants folded into the fused ops
            Q = mid("Q")
            nc.scalar.activation(out=Q[:], in_=uc[:], func=AF.Square,
                                 bias=bm125[:], scale=1.0)
            wa = mid("wa")
            nc.scalar.activation(out=wa[:], in_=uc[:], func=AF.Identity,
                                 bias=b5[:], scale=-4.0)

            pa1 = mid("pa1")  # (Q + 0.625/6)*uc
            nc.vector.scalar_tensor_tensor(out=pa1[:], in0=Q[:], scalar=float(0.625 / 6),
                                           in1=uc[:], op0=OP.add, op1=OP.mult)
            pa = mid("pa")    # -6*pa1*u2 = (qb-10)*uc*u2
            nc.vector.scalar_tensor_tensor(out=pa[:], in0=pa1[:], scalar=-6.0,
                                           in1=u2[:], op0=OP.mult, op1=OP.mult)
            vv = mid("vv")
            nc.vector.scalar_tensor_tensor(out=vv[:], in0=pa[:], scalar=1.0,
                                           in1=mf[:], op0=OP.add, op1=OP.mult)
            W1 = mid("W1")
            nc.vector.scalar_tensor_tensor(out=W1[:], in0=vv[:], scalar=1.0,
                                           in1=mf[:], op0=OP.add, op1=OP.subtract)
            wb = mid("wb"); nc.vector.tensor_mul(out=wb[:], in0=wa[:], in1=uc[:])
            s = mid("s")      # -6*Q + wb = (qb-10) + 0.625 + wb
            nc.vector.scalar_tensor_tensor(out=s[:], in0=Q[:], scalar=-6.0,
                                           in1=wb[:], op0=OP.mult, op1=OP.add)
            dv = mid("dv")    # (s - 0.625)*u2
            nc.vector.scalar_tensor_tensor(out=dv[:], in0=s[:], scalar=0.625,
                                           in1=u2[:], op0=OP.subtract, op1=OP.mult)
            im = mid("im"); nc.vector.tensor_mul(out=im[:], in0=tinr, in1=mf[:])
            Ct = mid("Ct")
            nc.vector.scalar_tensor_tensor(out=Ct[:], in0=dv[:], scalar=float(3 * DU),
                                           in1=im[:], op0=OP.mult, op1=OP.mult)
            H = mid("H"); nc.vector.tensor_mul(out=H[:], in0=i2[:], in1=vv[:])
            E = mid("E"); nc.vector.tensor_sub(out=E[:], in0=H[:], in1=Ct[:])
            Ft = mid("Ft"); nc.vector.tensor_add(out=Ft[:], in0=H[:], in1=E[:])
            U = mid("U"); nc.vector.tensor_mul(out=U[:], in0=i2[:], in1=Ft[:])
            V = mid("V"); nc.vector.tensor_mul(out=V[:], in0=i2[:], in1=W1[:])

            # wide triple helpers: [128,3w] views / free-dim step-0 broadcasts
            def as3(ap3w):
                return ap3w.rearrange("p (o w) -> p o w", o=3)

            def bc3(ap1w):
                return ap1w.rearrange("p (o w) -> p o w", o=1).to_broadcast([128, 3, w])

            dR3 = as3(inb[:, 0:3 * w])
            rstd123 = as3(st[:, w:4 * w])

            inrr0 = mid("inrr0")
            nc.vector.tensor_mul(out=inrr0[:], in0=tinr, in1=rstdf[0])
            dRsw = pmid.tile([128, 3 * w], F32, tag="dRsw", name=f"dRsw_{ic}")
            nc.vector.tensor_mul(out=as3(dRsw[:]), in0=dR3, in1=rstd123)
            dRs = [dRsw[:, j * w:(j + 1) * w] for j in range(3)]
            S0 = mid("S0"); nc.vector.tensor_mul(out=S0[:], in0=inrr0[:], in1=E[:])

            # Ri channels: DVE writes the raw products (one [128,w] + one wide
            # [128,3w]), PE runs identity @ raw then accumulates the rank-1
            # (-1s) x (davg*rstd) term in PSUM, ACT copies PSUM -> output slab.
            rw0 = mid("rw0")
            nc.vector.tensor_mul(out=rw0[:], in0=inrr0[:], in1=vv[:])
            rww = pmid.tile([128, 3 * w], F32, tag="rww", name=f"rww_{ic}")
            nc.vector.tensor_mul(out=as3(rww[:]), in0=as3(dRsw[:]), in1=bc3(V[:]))
            for c in range(4):
                rw = rw0 if c == 0 else rww[:, (c - 1) * w:c * w]
                pt = ppsum.tile([128, w], F32, tag=f"ps{c}", name=f"ps{c}_{ic}")
                nc.tensor.matmul(pt[:], ident, rw[:], start=True, stop=False)
                nc.tensor.matmul(pt[:], cneg[:], st[0:1, (4 + c) * w:(5 + c) * w],
                                 start=False, stop=True)
                nc.scalar.copy(out=och[f"ri{c}"], in_=pt[:])

            # Rd row 0: one wide op, S0 broadcast over k
            nc.vector.tensor_mul(out=as3(outb[:, 4 * w:7 * w]), in0=dR3, in1=bc3(S0[:]))

            # A_j = dRs_j * U (U broadcast over j), one wide op
            Aw = pmid.tile([128, 3 * w], F32, tag="Aw", name=f"Aw_{ic}")
            nc.vector.tensor_mul(out=as3(Aw[:]), in0=as3(dRsw[:]), in1=bc3(U[:]))
            # hr_j = H * rstd_j, one wide op
            hrw = pmid.tile([128, 3 * w], F32, tag="hrw", name=f"hrw_{ic}")
            nc.vector.tensor_mul(out=as3(hrw[:]), in0=bc3(H[:]), in1=rstd123)

            for j in range(3):
                # Rd row j+1: A_j broadcast over k against contiguous dR
                blk = outb[:, (7 + 3 * j) * w:(10 + 3 * j) * w]
                nc.vector.tensor_mul(out=as3(blk),
                                     in0=bc3(Aw[:, j * w:(j + 1) * w]), in1=dR3)
            # diagonal -= H*rstd_j: one op over the stride-4w diagonal views
            dg = outb[:, 7 * w:16 * w].rearrange("p (o w) -> p o w", o=9)[:, 0:9:4, :]
            nc.vector.tensor_sub(out=dg, in0=dg, in1=as3(hrw[:]))

            nc.sync.dma_start(out=outs["outb"][ic], in_=outb[:])


def _split_multiwaits(nc):
    """Walrus codegen can encode only one valued semaphore wait per
    instruction (the EVENTS semaphore_value field is shared). Tile sometimes
    emits more. Move extra valued waits onto injected same-engine NoOps placed
    immediately before the instruction — engines execute in order, so waiting
    earlier on the same queue is semantically identical."""
    skip = ("InstEventSemaphore",)
    for fn in nc.m.functions:
        for bb in fn.blocks:
            newlist = []
            changed = False
            for ins in bb.instructions:
                si = ins.sync_info
                if si is not None and type(ins).__name__ not in skip:
                    waits = list(si.on_wait or [])
                    valued = [w for w in waits if w.wait_value is not None]
                    if len(valued) > 1:
                        keep = valued[-1]
                        unvalued = [w for w in waits if w.wait_value is None]
                        for w in valued[:-1]:
                            nop = mybir.InstNoOp(
                                name=nc.get_next_instruction_name(), ins=[], outs=[])
                            nop.engine = ins.engine
                            nop.sync_info = mybir.SyncInfo(on_wait=[w], on_update=[])
                            newlist.append(nop)
                        ins.sync_info = mybir.SyncInfo(
                            on_wait=unvalued + [keep],
                            on_update=list(si.on_update or []))
                        changed = True
                newlist.append(ins)
            if changed:
                bb.instructions[:] = newlist


_NC = {}


def _build(split=True):
    if split in _NC:
        return _NC[split]
    nc = bass.Bass("TRN2", target_bir_lowering=False, debug=False)
    ins = {
        "inb": nc.dram_tensor("inb", [C, 128, 5 * W], F32, kind="ExternalInput").ap(),
        "msk": nc.dram_tensor("msk", [C, 128, W], U8, kind="ExternalInput").ap(),
        "stats": nc.dram_tensor("stats", [128, 8 * W + 128], F32, kind="ExternalInput").ap(),
    }
    outs = {
        "outb": nc.dram_tensor("outb", [C, 128, NCH * W], F32, kind="ExternalOutput").ap(),
    }
    with TileContext(nc) as tc:
        _dp_core_kernel(tc, outs, ins, C, W)
    if split:
        _split_multiwaits(nc)
    _NC[split] = nc
    return nc


def _build_timed(iters):
    """Variant that repeats the whole per-core workload `iters` times in a
    hardware For_i loop — used by the timing probe to isolate device exec
    time from dispatch/transfer overhead."""
    nc = bass.Bass("TRN2", target_bir_lowering=False, debug=False)
    ins = {
        "inb": nc.dram_tensor("inb", [C, 128, 5 * W], F32, kind="ExternalInput").ap(),
        "msk": nc.dram_tensor("msk", [C, 128, W], U8, kind="ExternalInput").ap(),
        "stats": nc.dram_tensor("stats", [128, 8 * W + 128], F32, kind="ExternalInput").ap(),
    }
    outs = {
        "outb": nc.dram_tensor("outb", [C, 128, NCH * W], F32, kind="ExternalOutput").ap(),
    }
    with TileContext(nc) as tc:
        with tc.For_i(0, iters, 1):
            _dp_core_kernel(tc, outs, ins, C, W)
    _split_multiwaits(nc)
    return nc


def _pack_plane(plane):
    """[ROWS, M] -> [C, 128, W]."""
    return plane.reshape(C, R, 128, M).transpose(0, 2, 1, 3).reshape(C, 128, W)


def _unpack_plane(packed):
    """[C, 128, W] -> [ROWS, M]."""
    return packed.reshape(C, 128, R, M).transpose(0, 2, 1, 3).reshape(ROWS, M)


def _pack_stats(davg, dstd):
    """davg/dstd [2, 800] -> [128, 8W] (rstd c=0..3 then dar c=0..3; slot r = type r%2)."""
    f32 = np.float32
    davg = davg.reshape(2, M, 4).astype(f32)
    dstd = dstd.reshape(2, M, 4).astype(f32)
    rstd = (f32(1.0) / dstd).astype(f32)
    dar = (davg * rstd).astype(f32)
    planes = np.empty((8, W), dtype=f32)
    for c in range(4):
        for r in range(R):
            t = r % 2
            planes[c, r * M:(r + 1) * M] = rstd[t, :, c]
            planes[4 + c, r * M:(r + 1) * M] = dar[t, :, c]
    out = np.empty((128, 8 * W + 128), dtype=f32)
    out[:, :8 * W] = planes.reshape(1, 8 * W)
    out[:, 8 * W:] = np.eye(128, dtype=f32)
    return np.ascontiguousarray(out)


def kernel(image_dR, x, Ri_xyz, mask, inr, davg, dstd, natoms_per_type):
    global LAST_RESULT
    image_dR = np.asarray(image_dR, dtype=np.float32)
    x = np.asarray(x, dtype=np.float32)
    inr = np.asarray(inr, dtype=np.float32)
    mask_u8 = np.asarray(mask).astype(np.uint8)
    davg = np.asarray(davg, dtype=np.float32)
    dstd = np.asarray(dstd, dtype=np.float32)

    nc = _build()
    stats = _pack_stats(davg, dstd)

    in_maps = []
    for ci in range(NCORES):
        f0 = ci * FPC
        dRc = image_dR[f0:f0 + FPC].reshape(ROWS, M, 3)
        planes = [
            _pack_plane(np.ascontiguousarray(dRc[:, :, 0])),
            _pack_plane(np.ascontiguousarray(dRc[:, :, 1])),
            _pack_plane(np.ascontiguousarray(dRc[:, :, 2])),
            _pack_plane(x[f0:f0 + FPC].reshape(ROWS, M)),
            _pack_plane(inr[f0:f0 + FPC].reshape(ROWS, M)),
        ]
        inb = np.ascontiguousarray(np.concatenate(planes, axis=2))
        in_maps.append({
            "inb": inb,
            "msk": np.ascontiguousarray(_pack_plane(mask_u8[f0:f0 + FPC].reshape(ROWS, M))),
            "stats": stats,
        })

    res = run_bass_kernel_spmd(nc, in_maps, core_ids=list(range(NCORES)))
    LAST_RESULT = res

    Ri = np.empty((B, N, M, 4), dtype=np.float32)
    Ri_d = np.empty((B, N, M, 4, 3), dtype=np.float32)
    for ci in range(NCORES):
        f0 = ci * FPC
        outb = res.results[ci]["outb"]
        for i, nm in enumerate(OUT_NAMES):
            pl = _unpack_plane(outb[:, :, i * W:(i + 1) * W]).reshape(FPC, N, M)
            if nm.startswith("ri"):
                Ri[f0:f0 + FPC, :, :, int(nm[2])] = pl
            else:
                Ri_d[f0:f0 + FPC, :, :, int(nm[1]), int(nm[2])] = pl
    return Ri, Ri_d
